# revision 1
# baseline (speedup 1.0000x reference)
"""DockPointNet forward loss on 8 Trainium2 NeuronCores — v2.

Key changes vs v1 (v1 was GpSimd-bound at 3.3ms of SWDGE descriptor gen,
~8.5ns per gathered row):
 - dst-side (c) per-edge gathers eliminated: c tables are SBUF-resident and
   per-edge c comes from a PE one-hot gather (ohT @ (-c_win)), window-uniform
   per tile so the program stays core-invariant.
 - GCN norm (dinv_s*dinv_d) folded into edge_attr on the host; no dinv column
   gathered. GCN + conv1 self-loops computed densely from local tables.
 - conv1 + GCN share one PSUM node accumulator (per-edge folding of g /
   dinv_d into the scatter rhs); start/stop flags replace zero-fill matmuls.
 - one-hot builds, LayerNorms and finalizes batched ([P,TB,P] / [P,NW,F]
   single ops); relu/square/psum-copies offloaded to the Scalar engine
   (activation accum_out gives relu+row-sum in one op).
 - all big streams shipped bf16 from the host (no on-chip f32->bf16 passes).
"""
import sys
from contextlib import ExitStack

import numpy as np

sys.path.insert(0, "/opt/trn_rl_repo")

NC = 8
P = 128
TB = 8
N_ATOMS = 50000
N_RES = 6250
N_G = 50048
NWG = N_G // P
EPS = 1e-5


def _bf16(a):
    import ml_dtypes
    return np.asarray(a, np.float32).astype(ml_dtypes.bfloat16)


# ======================================================================
# host-side preprocessing
# ======================================================================

def _build_partition(residue_index):
    base, rem = divmod(N_RES, NC)
    r_lo = [0]
    for k in range(NC):
        r_lo.append(r_lo[-1] + base + (1 if k < rem else 0))
    n_lo = [int(np.searchsorted(residue_index, r)) for r in r_lo]
    n_lo[-1] = residue_index.shape[0]
    return r_lo, n_lo


def _bucket(src, dst, owner_of_dst, dloc_of_dst, nwin, chunk_bounds,
            payload=None, force_min_tile=True):
    """Bucket edges by owner(dst) and dst-window; within a window sort by
    src (DRAM locality).  Tiles per window = cross-core max; each chunk's
    tile count padded to a TB multiple with all-pad tiles (window =
    chunk lo).  Returns per-core [S, DREL, PL], tiles_w list."""
    own = owner_of_dst[dst]
    dloc = dloc_of_dst[dst]
    percore = []
    counts = np.zeros((NC, nwin), np.int64)
    for k in range(NC):
        m = own == k
        s, dl = src[m], dloc[m]
        pl = (payload[m] if payload is not None else np.zeros_like(s))
        win = dl // P
        order = np.lexsort((s, win))
        s, dl, pl, win = s[order], dl[order], pl[order], win[order]
        counts[k] = np.bincount(win, minlength=nwin)
        percore.append((s, dl - win * P, pl, win))
    tw = [int(-(-counts[:, w].max() // P)) for w in range(nwin)]
    if force_min_tile:
        tw = [max(1, t) for t in tw]
    tiles_w = []
    tile_of_w = {}
    for (lo, hi) in chunk_bounds:
        tl = []
        for w in range(lo, hi):
            tile_of_w[w] = len(tiles_w) + len(tl)
            tl += [w] * tw[w]
        tl += [lo] * ((-len(tl)) % TB)
        tiles_w += tl
    T = len(tiles_w)
    out = []
    for k in range(NC):
        s, dr, pl, win = percore[k]
        S = np.zeros(T * P, np.int64)
        D = np.full(T * P, -1, np.int64)
        PL = np.zeros(T * P, pl.dtype)
        for w in range(nwin):
            sel = win == w
            cnt = int(sel.sum())
            b = tile_of_w[w] * P
            S[b:b + cnt] = s[sel]
            D[b:b + cnt] = dr[sel]
            PL[b:b + cnt] = pl[sel]
        out.append([S, D, PL])
    return out, tiles_w


def _flags(streams):
    """streams: tiles_w lists in execution order sharing one accumulator.
    Returns [(start_bool[], stop_bool[]), ...] per stream."""
    first, last = {}, {}
    for si, twl in enumerate(streams):
        for i, w in enumerate(twl):
            if w not in first:
                first[w] = (si, i)
            last[w] = (si, i)
    outs = [(np.zeros(len(t), bool), np.zeros(len(t), bool)) for t in streams]
    for w, (si, i) in first.items():
        outs[si][0][i] = True
    for w, (si, i) in last.items():
        outs[si][1][i] = True
    return outs


def _idx_cols(arr, tb=TB):
    T = arr.shape[0] // P
    return np.ascontiguousarray(arr.reshape(T // tb, tb, P).transpose(0, 2, 1))


def _node_major(arr, nw):
    a = arr.reshape(nw, P, *arr.shape[1:])
    return np.ascontiguousarray(np.swapaxes(a, 0, 1))


def host_prep(inputs):
    inp = {k: np.asarray(v) for k, v in inputs.items()}
    residue_index = inp["residue_index"].astype(np.int64)
    r_lo, n_lo = _build_partition(residue_index)
    RW = -(-max(r_lo[i + 1] - r_lo[i] for i in range(NC)) // P)
    RLOC = RW * P
    RTOT = NC * RLOC

    # --- atom layout: residue-window groups padded uniformly across cores
    percore_ridx = []
    cnts = np.zeros((NC, RW), np.int64)
    for k in range(NC):
        ridx = residue_index[n_lo[k]:n_lo[k + 1]] - r_lo[k]
        percore_ridx.append(ridx)
        cnts[k] = np.bincount(ridx // P, minlength=RW)
    tw_pool = [max(1, int(-(-cnts[:, w].max() // P))) for w in range(RW)]
    tile_rw = []
    for w in range(RW):
        tile_rw += [w] * tw_pool[w]
    NW = len(tile_rw)
    NLOC = NW * P
    wbase = np.cumsum([0] + [t * P for t in tw_pool])[:RW]

    owner = np.zeros(N_ATOMS, np.int64)
    dloc_pad = np.zeros(N_ATOMS, np.int64)
    lay = []
    for k in range(NC):
        ridx = percore_ridx[k]
        nloc = np.full(len(ridx), -1, np.int64)
        for w in range(RW):
            sel = np.nonzero(ridx // P == w)[0]
            nloc[sel] = wbase[w] + np.arange(len(sel))
        owner[n_lo[k]:n_lo[k + 1]] = k
        dloc_pad[n_lo[k]:n_lo[k + 1]] = nloc
        lay.append(nloc)

    CMID = (NW + 1) // 2
    chunks = [(0, CMID), (CMID, NW)]

    # --- conv1 edges (radius graph, self-loops handled densely)
    s1 = inp["rad_edge_index"][0].astype(np.int64)
    d1 = inp["rad_edge_index"][1].astype(np.int64)
    c1, tiles_w1 = _bucket(s1, d1, owner, dloc_pad, NW, chunks)
    T1 = len(tiles_w1)
    G1 = T1 // TB
    GSPL1 = len([w for w in tiles_w1 if w < CMID])  # chunk0 tiles
    # (chunk0 tiles are a prefix and padded to TB)
    GSPL1 = GSPL1 // TB if GSPL1 % TB == 0 else (GSPL1 + (-GSPL1) % TB) // TB

    # --- gcn edges (bond graph, self-loops dense)
    s2 = inp["edge_index"][0].astype(np.int64)
    d2 = inp["edge_index"][1].astype(np.int64)
    nb = s2.shape[0]
    eid = np.arange(nb)
    c2, tiles_w2 = _bucket(s2, d2, owner, dloc_pad, NW, chunks, payload=eid)
    T2 = len(tiles_w2)
    G2 = T2 // TB
    GSPL2 = len([w for w in tiles_w2 if w < CMID])
    GSPL2 = GSPL2 // TB if GSPL2 % TB == 0 else (GSPL2 + (-GSPL2) % TB) // TB

    # degrees (include self-loops, as the reference does)
    deg1_g = (np.bincount(d1, minlength=N_ATOMS) + 1).astype(np.float32)
    deg2_g = (np.bincount(d2, minlength=N_ATOMS) + 1).astype(np.float32)
    dinv2_g = deg2_g ** -0.5

    # --- conv2 edges (residue radius graph, self-loops as edges)
    rloops = np.arange(N_RES)
    s3 = np.concatenate([inp["res_rad_edge_index"][0], rloops]).astype(np.int64)
    d3 = np.concatenate([inp["res_rad_edge_index"][1], rloops]).astype(np.int64)
    r_owner = np.zeros(N_RES, np.int64)
    r_locid = np.zeros(N_RES, np.int64)
    for k in range(NC):
        r_owner[r_lo[k]:r_lo[k + 1]] = k
        r_locid[r_lo[k]:r_lo[k + 1]] = np.arange(r_lo[k + 1] - r_lo[k])
    r_padg = r_owner * RLOC + r_locid
    deg3_g = np.bincount(d3, minlength=N_RES).astype(np.float32)
    c3, tiles_w3 = _bucket(r_padg[s3], d3, r_owner, r_locid, RW, [(0, RW)])
    T3 = len(tiles_w3)
    G3 = T3 // TB

    # --- flags (conv1 + gcn share one accumulator; conv2 its own)
    (st1, sp1), (st2, sp2) = _flags([tiles_w1, tiles_w2])
    (st3, sp3), = _flags([tiles_w3])
    (stp, spp), = _flags([tile_rw])

    # --- global tables
    xcatT = np.zeros((34, N_G), np.float32)
    xcatT[:30, :N_ATOMS] = inp["x"].astype(np.float32).T
    xcatT[30:33, :N_ATOMS] = inp["pos"].astype(np.float32).T
    xcatT[33, :] = 1.0
    deg2_gt = np.ones(N_G, np.float32)
    deg2_gt[:N_ATOMS] = deg2_g

    # --- weights
    w_pc1 = inp["w_pc1"].astype(np.float32)
    w_gcn = inp["w_gcn"].astype(np.float32)
    w_pro = np.zeros((34, 128), np.float32)
    w_pro[:33, :64] = w_pc1
    w_pro[33, :64] = inp["b_pc1"]
    w_pro[:30, 64:] = w_gcn[:30]
    w_c1n = -np.ascontiguousarray(w_pc1[30:33])
    w_ea = np.ascontiguousarray(w_gcn[30:42])
    w_rc = inp["w_rc"].astype(np.float32)
    w2a = np.zeros((68, 128), np.float32)
    w2a[:64] = w_rc[:64]
    w2a[64:67] = w_rc[64:67]
    w2a[67] = inp["b_rc"]
    w2cn = np.zeros((68, 128), np.float32)
    w2cn[64:67] = -w_rc[64:67]

    # --- loss pairs
    y = inp["y_lab"].astype(np.int64)
    pos_w = float((y == 0).sum()) / float((y == 1).sum())
    ppc = len(y) // NC
    PPAD = -(-ppc // (P * 4)) * (P * 4)
    src_g = r_padg[inp["src_idx"].astype(np.int64)]
    tgt_g = r_padg[inp["tgt_idx"].astype(np.int64)]

    dims = dict(RW=RW, RLOC=RLOC, NW=NW, NLOC=NLOC, CMID=CMID,
                T1=T1, G1=G1, GSPL1=GSPL1, T2=T2, G2=G2, GSPL2=GSPL2,
                T3=T3, G3=G3, TP=PPAD // P,
                tiles_w1=tiles_w1, tiles_w2=tiles_w2, tiles_w3=tiles_w3,
                tile_rw=tile_rw,
                st1=st1, sp1=sp1, st2=st2, sp2=sp2, st3=st3, sp3=sp3,
                stp=stp, spp=spp)

    in_maps = []
    pos_f = inp["pos"].astype(np.float32)
    ea_f = inp["edge_attr"].astype(np.float32)
    for k in range(NC):
        n0, n1 = n_lo[k], n_lo[k + 1]
        nloc = lay[k]
        # local-layout tables
        xcl = np.zeros((34, NLOC), np.float32)
        xcl[:30, nloc] = inp["x"].astype(np.float32)[n0:n1].T
        xcl[30:33, nloc] = pos_f[n0:n1].T
        xcl[33, nloc] = 1.0
        posl = np.zeros((NLOC, 3), np.float32)
        posl[nloc] = pos_f[n0:n1]
        d1l = np.zeros(NLOC, np.float32)
        d1l[nloc] = deg1_g[n0:n1]
        d2l = np.ones(NLOC, np.float32)
        d2l[nloc] = deg2_g[n0:n1]
        rr = np.full(NLOC, -1, np.int64)
        rr[nloc] = (residue_index[n0:n1] - r_lo[k]) % P
        d3l = np.zeros(RLOC, np.float32)
        d3l[:r_lo[k + 1] - r_lo[k]] = deg3_g[r_lo[k]:r_lo[k + 1]]

        # gcn per-slot payload: ea * dinv_s * dinv_d  (pad slots: 0)
        S2, D2, PL2 = c2[k]
        real2 = D2 >= 0
        eaP = np.zeros((T2 * P, 12), np.float32)
        dv2 = np.zeros(T2 * P, np.float32)
        if real2.any():
            e = PL2[real2]
            eaP[real2] = (ea_f[e] * (dinv2_g[s2[e]] * dinv2_g[d2[e]])[:, None])
            dv2[real2] = dinv2_g[d2[e]]

        lo, hi = k * ppc, (k + 1) * ppc
        psrc = np.zeros(PPAD, np.int64)
        ptgt = np.zeros(PPAD, np.int64)
        mpv = np.zeros(PPAD, np.float32)
        mnv = np.zeros(PPAD, np.float32)
        psrc[:hi - lo] = src_g[lo:hi]
        ptgt[:hi - lo] = tgt_g[lo:hi]
        yk = y[lo:hi]
        mpv[:hi - lo] = (yk == 1) * (-pos_w / len(y))
        mnv[:hi - lo] = (yk == 0) * (1.0 / len(y))

        vec = lambda n: inp[n].astype(np.float32).reshape(1, -1)
        m = dict(
            xcatT=_bf16(xcatT),
            xcat_locT=_bf16(xcl),
            pos_locT=_bf16(posl.T),
            pos_nm=_node_major(posl, NW),
            deg2g=_node_major(deg2_gt, NWG),
            w_pro=_bf16(w_pro), w_c1n=_bf16(w_c1n), w_ea=_bf16(w_ea),
            w_ae=_bf16(inp["w_ae"]), w_re=_bf16(inp["w_re"]),
            w_rg=_bf16(inp["w_rg"]),
            w2a=_bf16(w2a), w2cn=_bf16(w2cn),
            b_ae=vec("b_ae"), b_re=vec("b_re"), b_rg=vec("b_rg"),
            g_pc1=vec("g_pc1"), be_pc1=vec("be_pc1"), b_gcn=vec("b_gcn"),
            g_ae=vec("g_ae"), be_ae=vec("be_ae"),
            g_re=vec("g_re"), be_re=vec("be_re"),
            g_rc=vec("g_rc"), be_rc=vec("be_rc"),
            g_rg=vec("g_rg"), be_rg=vec("be_rg"),
            e1_src=_idx_cols(c1[k][0]).astype(np.int32),
            e1_drel=_idx_cols(c1[k][1]).astype(np.int8),
            e1_drelT=_bf16(c1[k][1].reshape(G1, TB * P)),
            e2_src=_idx_cols(S2).astype(np.int32),
            e2_drel=_idx_cols(D2).astype(np.int8),
            dinvd2=_idx_cols(dv2),
            eaT=_bf16(eaP.T),
            e3_src=_idx_cols(c3[k][0]).astype(np.int32),
            e3_drel=_idx_cols(c3[k][1]).astype(np.int8),
            e3_drelT=_bf16(c3[k][1].reshape(G3, TB * P)),
            deg1_loc=_node_major(d1l, NW),
            deg2_loc=_node_major(d2l, NW),
            deg3_loc=_node_major(d3l, RW),
            res_rel=_node_major(rr.astype(np.int8), NW),
            pr_src=_idx_cols(psrc, 4).astype(np.int32),
            pr_tgt=_idx_cols(ptgt, 4).astype(np.int32),
            mpn=_idx_cols(mpv, 4),
            mnn=_idx_cols(mnv, 4),
        )
        in_maps.append(m)
    return in_maps, dims


# ======================================================================
# device program
# ======================================================================

def build_program(dims):
    import concourse.bass as bass
    import concourse.tile as tile
    from concourse import mybir
    from concourse.bass import IndirectOffsetOnAxis
    from concourse.masks import make_identity
    from concourse import bacc

    dt = mybir.dt
    Alu = mybir.AluOpType
    Act = mybir.ActivationFunctionType
    AX = mybir.AxisListType.X
    RW, RLOC, NW, NLOC = dims["RW"], dims["RLOC"], dims["NW"], dims["NLOC"]
    CMID = dims["CMID"]
    T1, G1, GSPL1 = dims["T1"], dims["G1"], dims["GSPL1"]
    T2, G2, GSPL2 = dims["T2"], dims["G2"], dims["GSPL2"]
    T3, G3, TP = dims["T3"], dims["G3"], dims["TP"]
    tiles_w1, tiles_w2 = dims["tiles_w1"], dims["tiles_w2"]
    tiles_w3, tile_rw = dims["tiles_w3"], dims["tile_rw"]
    st1, sp1 = dims["st1"], dims["sp1"]
    st2, sp2 = dims["st2"], dims["sp2"]
    st3, sp3 = dims["st3"], dims["sp3"]
    stp_f, spp_f = dims["stp"], dims["spp"]
    GP = TP // 4
    RTOT = NC * RLOC
    NGL = NW // TB + (1 if NW % TB else 0)

    nc = bacc.Bacc("TRN2", target_bir_lowering=False, debug=False,
                   num_devices=NC)
    f32, bf16, i32, i8 = dt.float32, dt.bfloat16, dt.int32, dt.int8

    def param(name, shape, dtp, out=False):
        return nc.declare_dram_parameter(name, list(shape), dtp, isOutput=out)

    xcatT = param("xcatT", (34, N_G), bf16)
    xcat_locT = param("xcat_locT", (34, NLOC), bf16)
    pos_locT = param("pos_locT", (3, NLOC), bf16)
    pos_nm = param("pos_nm", (P, NW, 3), f32)
    deg2g = param("deg2g", (P, NWG), f32)
    w_pro = param("w_pro", (34, 128), bf16)
    w_c1n = param("w_c1n", (3, 64), bf16)
    w_ea = param("w_ea", (12, 64), bf16)
    w_ae = param("w_ae", (64, 64), bf16)
    w_re = param("w_re", (64, 64), bf16)
    w_rg = param("w_rg", (128, 128), bf16)
    w2a = param("w2a", (68, 128), bf16)
    w2cn = param("w2cn", (68, 128), bf16)
    vnames64 = ["b_ae", "b_re", "g_pc1", "be_pc1", "b_gcn",
                "g_ae", "be_ae", "g_re", "be_re"]
    vnames128 = ["b_rg", "g_rc", "be_rc", "g_rg", "be_rg"]
    vecs = {n: param(n, (1, 64), f32) for n in vnames64}
    vecs.update({n: param(n, (1, 128), f32) for n in vnames128})
    e1_src = param("e1_src", (G1, P, TB), i32)
    e1_drel = param("e1_drel", (G1, P, TB), i8)
    e1_drelT = param("e1_drelT", (G1, TB * P), bf16)
    e2_src = param("e2_src", (G2, P, TB), i32)
    e2_drel = param("e2_drel", (G2, P, TB), i8)
    dinvd2 = param("dinvd2", (G2, P, TB), f32)
    eaT = param("eaT", (12, T2 * P), bf16)
    e3_src = param("e3_src", (G3, P, TB), i32)
    e3_drel = param("e3_drel", (G3, P, TB), i8)
    e3_drelT = param("e3_drelT", (G3, TB * P), bf16)
    deg1_loc = param("deg1_loc", (P, NW), f32)
    deg2_loc = param("deg2_loc", (P, NW), f32)
    deg3_loc = param("deg3_loc", (P, RW), f32)
    res_rel = param("res_rel", (P, NW), i8)
    pr_src = param("pr_src", (GP, P, 4), i32)
    pr_tgt = param("pr_tgt", (GP, P, 4), i32)
    mpn = param("mpn", (GP, P, 4), f32)
    mnn = param("mnn", (GP, P, 4), f32)
    loss_part = param("loss_part", (1, 1), f32, out=True)

    table_g = nc.dram_tensor("table_g", [N_G, 128], bf16)
    table2_l = nc.dram_tensor("table2_l", [RLOC, 128], bf16)
    table2 = nc.dram_tensor("table2", [RTOT, 128], bf16,
                            addr_space="Shared")
    x3_l = nc.dram_tensor("x3_l", [RLOC, 128], bf16)
    x3_a = nc.dram_tensor("x3_a", [RTOT, 128], bf16, addr_space="Shared")

    def bc_mid(ap_, reps):
        return bass.AP(tensor=ap_.tensor, offset=ap_.offset,
                       ap=[ap_.ap[0], [0, reps], ap_.ap[1]])

    def bc_inner(ap_, inner):
        return bass.AP(tensor=ap_.tensor, offset=ap_.offset,
                       ap=[ap_.ap[0], ap_.ap[1], [0, inner]])

    def bc2(ap_, a, b):
        return bass.AP(tensor=ap_.tensor, offset=ap_.offset,
                       ap=[ap_.ap[0], [0, a], [0, b]])

    def row_bcast(src2d):
        # [1, N] dram AP -> [P, N] partition-broadcast
        return bass.AP(tensor=src2d.tensor, offset=src2d.offset,
                       ap=[[0, P], src2d.ap[-1]])

    with tile.TileContext(nc) as tc, ExitStack() as ctx:
        consts = ctx.enter_context(tc.tile_pool(name="consts", bufs=1))
        iota16 = consts.tile([P, P], dt.int16)
        nc.gpsimd.iota(iota16[:], pattern=[[1, P]], base=0,
                       channel_multiplier=0)
        iotab = consts.tile([P, P], bf16)
        nc.vector.tensor_copy(iotab[:], iota16[:])
        iota8 = consts.tile([P, P], i8)
        nc.vector.tensor_copy(iota8[:], iota16[:])
        iotp16 = consts.tile([P, 1], dt.int16)
        nc.gpsimd.iota(iotp16[:], pattern=[[0, 1]], base=0,
                       channel_multiplier=1)
        iotaP1 = consts.tile([P, 1], bf16)
        nc.vector.tensor_copy(iotaP1[:], iotp16[:])
        iotaP8 = consts.tile([P, 1], i8)
        nc.vector.tensor_copy(iotaP8[:], iotp16[:])
        ident = consts.tile([P, P], bf16)
        make_identity(nc, ident[:])
        epst = consts.tile([P, 1], f32)
        nc.vector.memset(epst[:], EPS)
        eps30 = consts.tile([P, 1], f32)
        nc.vector.memset(eps30[:], 1e-30)
        ones_col = consts.tile([P, 1], f32)
        nc.vector.memset(ones_col[:], 1.0)
        ones1p = consts.tile([1, P], bf16)
        nc.vector.memset(ones1p[:], 1.0)

        wpool = ctx.enter_context(tc.tile_pool(name="weights", bufs=1))

        def wload(pp, shape, tag):
            t = wpool.tile(list(shape), bf16, tag=tag)
            nc.sync.dma_start(out=t[:], in_=pp[:, :])
            return t

        w_pro_b = wload(w_pro, (34, 128), "wpro")
        w_c1n_b = wload(w_c1n, (3, 64), "wc1n")
        w_ea_b = wload(w_ea, (12, 64), "wea")
        w_ae_b = wload(w_ae, (64, 64), "wae")
        w_re_b = wload(w_re, (64, 64), "wre")
        w_rg_b = wload(w_rg, (128, 128), "wrg")
        w2a_b = wload(w2a, (68, 128), "w2a")
        w2cn_b = wload(w2cn, (68, 128), "w2cn")

        def bvec(name):
            src = vecs[name][:, :]
            d = src.shape[1]
            t = wpool.tile([P, d], f32, tag=f"bv_{name}")
            nc.sync.dma_start(out=t[:], in_=row_bcast(src))
            return t

        be_pc1_t, b_gcn_t = bvec("be_pc1"), bvec("b_gcn")
        b_ae_t, g_ae_t, be_ae_t = bvec("b_ae"), bvec("g_ae"), bvec("be_ae")
        b_re_t, g_re_t, be_re_t = bvec("b_re"), bvec("g_re"), bvec("be_re")
        b_rg_t, g_rg_t, be_rg_t = bvec("b_rg"), bvec("g_rg"), bvec("be_rg")
        be_rc_t = bvec("be_rc")
        g_pc1_f = bvec("g_pc1")
        g_rc_f = bvec("g_rc")
        g_pc1b = wpool.tile([P, 64], bf16, tag="gpc1b")
        nc.vector.tensor_copy(g_pc1b[:], g_pc1_f[:])
        g_rcb = wpool.tile([P, 128], bf16, tag="grcb")
        nc.vector.tensor_copy(g_rcb[:], g_rc_f[:])

        nlp = ctx.enter_context(tc.tile_pool(name="nloc", bufs=1))
        deg1_t = nlp.tile([P, NW], f32)
        nc.sync.dma_start(out=deg1_t[:], in_=deg1_loc[:, :])
        deg2l_t = nlp.tile([P, NW], f32)
        nc.sync.dma_start(out=deg2l_t[:], in_=deg2_loc[:, :])
        dinvl_t = nlp.tile([P, NW], f32)
        nc.scalar.activation(dinvl_t[:], deg2l_t[:], Act.Sqrt, scale=1.0)
        nc.vector.reciprocal(dinvl_t[:], dinvl_t[:])
        dinvl_sq = nlp.tile([P, NW], f32)
        nc.vector.tensor_tensor(out=dinvl_sq[:], in0=dinvl_t[:],
                                in1=dinvl_t[:], op=Alu.mult)
        deg3_t = nlp.tile([P, RW], f32)
        nc.sync.dma_start(out=deg3_t[:], in_=deg3_loc[:, :])
        rrel8 = nlp.tile([P, NW], i8)
        nc.sync.dma_start(out=rrel8[:], in_=res_rel[:, :])
        rrelb = nlp.tile([P, NW], bf16)
        nc.vector.tensor_copy(rrelb[:], rrel8[:])
        deg2g_t = nlp.tile([P, NWG], f32)
        nc.sync.dma_start(out=deg2g_t[:], in_=deg2g[:, :])
        dinvg_t = nlp.tile([P, NWG], f32)
        nc.scalar.activation(dinvg_t[:], deg2g_t[:], Act.Sqrt, scale=1.0)
        nc.vector.reciprocal(dinvg_t[:], dinvg_t[:])

        # persistent SBUF stages
        stp = ctx.enter_context(tc.tile_pool(name="stage", bufs=1))
        stage = stp.tile([P, NW, 64], bf16)
        poolrhs = stp.tile([P, NW, 68], bf16)
        a_loc = stp.tile([P, NW, 64], bf16)
        selfgcn = stp.tile([P, NW, 64], bf16)
        nc1sb = stp.tile([P, NW, 64], bf16)
        off = stp.tile([P, NW, 64], bf16)
        nt2c = stp.tile([P, RW, 128], bf16)
        q_all = stp.tile([P, T2, 64], bf16)
        x3c = stp.tile([P, RW, 128], bf16)
        h_all = stp.tile([P, NW, 64], bf16)
        x3sb = stp.tile([P, RW, 128], bf16)
        resdat_sb = stp.tile([P, RW, 68], bf16)

        # ---------- P1: global table [a1(64) | p~(64)]
        GTB = 16
        with tc.tile_pool(name="pro", bufs=3) as pro, \
             tc.tile_pool(name="prop", bufs=2, space="PSUM") as prop:
            ngrp = NWG // GTB + (1 if NWG % GTB else 0)
            for g in range(ngrp):
                jmax = min(GTB, NWG - g * GTB)
                xin = pro.tile([34, GTB * P], bf16, tag="xin")
                nc.scalar.dma_start(
                    out=xin[:, 0:jmax * P],
                    in_=xcatT[:, g * GTB * P:g * GTB * P + jmax * P])
                # two PSUM tiles (2 banks each) per 16-tile group
                for h in range(2):
                    hj = min(max(jmax - h * 8, 0), 8)
                    if hj == 0:
                        continue
                    ps = prop.tile([P, 8 * 128], f32, space="PSUM", tag="ps")
                    for j in range(hj):
                        jj = h * 8 + j
                        nc.tensor.matmul(ps[:, j * 128:(j + 1) * 128],
                                         lhsT=xin[:, jj * P:(jj + 1) * P],
                                         rhs=w_pro_b[:], start=True,
                                         stop=True, skip_group_check=True)
                    psv = ps[:].rearrange("p (t c) -> p t c", c=128)
                    ot = pro.tile([P, 8, 128], bf16, tag="ot")
                    t0 = g * GTB + h * 8
                    nc.vector.tensor_copy(ot[:, 0:hj, 0:64],
                                          psv[:, 0:hj, 0:64])
                    nc.vector.tensor_tensor(
                        out=ot[:, 0:hj, 64:128], in0=psv[:, 0:hj, 64:128],
                        in1=bc_inner(dinvg_t[:, t0:t0 + hj], 64),
                        op=Alu.mult)
                    nc.sync.dma_start(
                        out=table_g[t0 * P:(t0 + hj) * P, :]
                            .rearrange("(t p) c -> p t c", p=P),
                        in_=ot[:, 0:hj, :])

        # ---------- P1b: local a_loc / selfgcn;  P2: nc1sb;  P2b: q_all
        with tc.tile_pool(name="locp", bufs=3) as locp, \
             tc.tile_pool(name="locps", bufs=2, space="PSUM") as locps:
            for g in range(NGL):
                jmax = min(TB, NW - g * TB)
                pin = locp.tile([3, TB * P], bf16, tag="pin")
                nc.scalar.dma_start(
                    out=pin[:, 0:jmax * P],
                    in_=pos_locT[:, g * TB * P:g * TB * P + jmax * P])
                ps = locps.tile([P, TB * 64], f32, space="PSUM", tag="cps")
                for j in range(jmax):
                    nc.tensor.matmul(ps[:, j * 64:(j + 1) * 64],
                                     lhsT=pin[:, j * P:(j + 1) * P],
                                     rhs=w_c1n_b[:], start=True, stop=True,
                                     skip_group_check=True)
                nc.scalar.activation(
                    nc1sb[:, g * TB:g * TB + jmax, :],
                    ps[:].rearrange("p (t c) -> p t c", c=64)[:, 0:jmax, :],
                    Act.Copy, scale=1.0)

        # ---------- off = conv1-self + gcn-self + deg1*be1 + deg2*bgcn
        def emit_off():
          with tc.tile_pool(name="selfp", bufs=1) as sfp:
            A = sfp.tile([P, NW, 64], bf16, tag="zt")
            nc.vector.tensor_tensor(out=A[:], in0=a_loc[:], in1=nc1sb[:],
                                    op=Alu.add)
            nc.scalar.activation(A[:], A[:], Act.Relu, scale=1.0)
            sm = sfp.tile([P, NW], f32, tag="ssm")
            nc.vector.tensor_reduce(out=sm[:], in_=A[:], op=Alu.add, axis=AX)
            mu = sfp.tile([P, NW], f32, tag="smu")
            nc.vector.tensor_scalar(out=mu[:], in0=sm[:], scalar1=1.0 / 64,
                                    scalar2=None, op0=Alu.mult)
            B = sfp.tile([P, NW, 64], bf16, tag="szc")
            nc.vector.tensor_tensor(out=B[:], in0=A[:],
                                    in1=bc_inner(mu[:], 64), op=Alu.subtract)
            C = sfp.tile([P, NW, 64], bf16, tag="ssq3")
            nc.scalar.activation(C[:], B[:], Act.Square, scale=1.0)
            ssq = sfp.tile([P, NW], f32, tag="sss")
            nc.vector.tensor_reduce(out=ssq[:], in_=C[:], op=Alu.add, axis=AX)
            Av = sfp.tile([P, NW], f32, tag="sAv")
            nc.scalar.activation(Av[:], ssq[:], Act.Sqrt, bias=epst[:],
                                 scale=1.0 / 64)
            nc.vector.reciprocal(Av[:], Av[:])
            nc.vector.tensor_tensor(out=C[:], in0=B[:],
                                    in1=bc_inner(Av[:], 64), op=Alu.mult)
            nc.vector.tensor_tensor(out=off[:], in0=C[:],
                                    in1=bc_mid(g_pc1b[:], NW), op=Alu.mult)
            nc.vector.tensor_tensor(out=off[:], in0=off[:], in1=selfgcn[:],
                                    op=Alu.add)
            nc.vector.tensor_tensor(out=A[:], in0=bc_inner(deg1_t[:], 64),
                                    in1=bc_mid(be_pc1_t[:], NW), op=Alu.mult)
            nc.vector.tensor_tensor(out=off[:], in0=off[:], in1=A[:],
                                    op=Alu.add)
            nc.vector.tensor_tensor(out=A[:], in0=bc_inner(deg2l_t[:], 64),
                                    in1=bc_mid(b_gcn_t[:], NW), op=Alu.mult)
            nc.vector.tensor_tensor(out=off[:], in0=off[:], in1=A[:],
                                    op=Alu.add)

        # ---------- conv1 + gcn edge phase (shared PSUM accumulator)
        def conv1_groups(pool, gpool, zpool, bcpool, acc, g_lo, g_hi, ch_lo):
            for g in range(g_lo, g_hi):
                srcs = pool.tile([P, TB], i32, tag="srcs")
                nc.sync.dma_start(out=srcs[:], in_=e1_src[g, :, :])
                drel8 = pool.tile([P, TB], i8, tag="drel8")
                nc.sync.dma_start(out=drel8[:], in_=e1_drel[g, :, :])
                drow = pool.tile([1, TB * P], bf16, tag="drow")
                nc.sync.dma_start(out=drow[:], in_=e1_drelT[g:g + 1, :])
                asb = gpool.tile([P, TB, 64], bf16, tag="asb")
                for j in range(TB):
                    nc.gpsimd.indirect_dma_start(
                        out=asb[:, j, :], out_offset=None, in_=table_g[:, :],
                        in_offset=IndirectOffsetOnAxis(ap=srcs[:, j:j + 1],
                                                       axis=0),
                        element_offset=0)
                oh = pool.tile([P, TB, P], bf16, tag="oh")
                nc.vector.tensor_tensor(out=oh[:, :, :],
                                        in0=bc_mid(iota8[:], TB),
                                        in1=bc_inner(drel8[:], P),
                                        op=Alu.is_equal)
                dbc = bcpool.tile([P, TB * P], f32, space="PSUM",
                                  tag="dbc")
                nc.tensor.matmul(dbc[:, 0:512], lhsT=ones1p[:],
                                 rhs=drow[:, 0:512], start=True, stop=True,
                                 skip_group_check=True)
                nc.tensor.matmul(dbc[:, 512:1024], lhsT=ones1p[:],
                                 rhs=drow[:, 512:1024], start=True, stop=True,
                                 skip_group_check=True)
                ohT = pool.tile([P, TB, P], bf16, tag="ohT")
                nc.vector.tensor_tensor(
                    out=ohT[:, :, :], in0=bc2(iotaP1[:], TB, P),
                    in1=dbc[:].rearrange("p (a b) -> p a b", b=P),
                    op=Alu.is_equal)
                zps = zpool.tile([P, TB, 64], f32, space="PSUM", tag="zps")
                for j in range(TB):
                    t = g * TB + j
                    nc.tensor.matmul(zps[:, j, :], lhsT=ohT[:, j, :],
                                     rhs=nc1sb[:, tiles_w1[t], :],
                                     start=True, stop=False,
                                     skip_group_check=True)
                    nc.tensor.matmul(zps[:, j, :], lhsT=ident[:],
                                     rhs=asb[:, j, :], start=False, stop=True,
                                     skip_group_check=True)
                zr = pool.tile([P, TB, 64], bf16, tag="zr")
                nc.scalar.activation(zr[:], zps[:, :, :], Act.Relu, scale=1.0)
                sm = pool.tile([P, TB], f32, tag="sm")
                nc.vector.tensor_reduce(out=sm[:], in_=zr[:, :, :],
                                        op=Alu.add, axis=AX)
                mu = pool.tile([P, TB], f32, tag="mu")
                nc.vector.tensor_scalar(out=mu[:], in0=sm[:],
                                        scalar1=1.0 / 64, scalar2=None,
                                        op0=Alu.mult)
                zc = pool.tile([P, TB, 64], bf16, tag="zc")
                nc.vector.tensor_tensor(out=zc[:, :, :], in0=zr[:, :, :],
                                        in1=bc_inner(mu[:], 64),
                                        op=Alu.subtract)
                sqs = pool.tile([P, TB, 64], bf16, tag="sqs")
                nc.scalar.activation(sqs[:], zc[:], Act.Square, scale=1.0)
                ssq = pool.tile([P, TB], f32, tag="ssq")
                nc.vector.tensor_reduce(out=ssq[:], in_=sqs[:, :, :],
                                        op=Alu.add, axis=AX)
                Avt = pool.tile([P, TB], f32, tag="Av")
                nc.scalar.activation(Avt[:], ssq[:], Act.Sqrt, bias=epst[:],
                                     scale=1.0 / 64)
                nc.vector.reciprocal(Avt[:], Avt[:])
                t1 = pool.tile([P, TB, 64], bf16, tag="t1")
                nc.vector.tensor_tensor(out=t1[:, :, :], in0=zc[:, :, :],
                                        in1=bc_inner(Avt[:], 64),
                                        op=Alu.mult)
                rhs = pool.tile([P, TB, 64], bf16, tag="rhs")
                nc.vector.tensor_tensor(out=rhs[:, :, :], in0=t1[:, :, :],
                                        in1=bc_mid(g_pc1b[:], TB),
                                        op=Alu.mult)
                for j in range(TB):
                    t = g * TB + j
                    w = tiles_w1[t]
                    nc.tensor.matmul(
                        acc[:, (w - ch_lo) * 64:(w - ch_lo + 1) * 64],
                        lhsT=oh[:, j, :], rhs=rhs[:, j, :],
                        start=bool(st1[t]), stop=bool(sp1[t]),
                        skip_group_check=True)

        def gcn_groups(pool, gpool, acc, g_lo, g_hi, ch_lo):
            for g in range(g_lo, g_hi):
                srcs = pool.tile([P, TB], i32, tag="gsrcs")
                nc.sync.dma_start(out=srcs[:], in_=e2_src[g, :, :])
                drel8 = pool.tile([P, TB], i8, tag="gdrel8")
                nc.sync.dma_start(out=drel8[:], in_=e2_drel[g, :, :])
                dv = pool.tile([P, TB], f32, tag="gdv")
                nc.sync.dma_start(out=dv[:], in_=dinvd2[g, :, :])
                psb = gpool.tile([P, TB, 64], bf16, tag="psb")
                for j in range(TB):
                    nc.gpsimd.indirect_dma_start(
                        out=psb[:, j, :], out_offset=None, in_=table_g[:, :],
                        in_offset=IndirectOffsetOnAxis(ap=srcs[:, j:j + 1],
                                                       axis=0),
                        element_offset=64)
                oh = pool.tile([P, TB, P], bf16, tag="goh")
                nc.vector.tensor_tensor(out=oh[:, :, :],
                                        in0=bc_mid(iota8[:], TB),
                                        in1=bc_inner(drel8[:], P),
                                        op=Alu.is_equal)
                msg = pool.tile([P, TB, 64], bf16, tag="gmsg")
                nc.vector.tensor_tensor(out=msg[:, :, :], in0=psb[:, :, :],
                                        in1=bc_inner(dv[:], 64), op=Alu.mult)
                nc.vector.tensor_tensor(out=msg[:, :, :], in0=msg[:, :, :],
                                        in1=q_all[:, g * TB:(g + 1) * TB, :],
                                        op=Alu.add)
                for j in range(TB):
                    t = g * TB + j
                    w = tiles_w2[t]
                    nc.tensor.matmul(
                        acc[:, (w - ch_lo) * 64:(w - ch_lo + 1) * 64],
                        lhsT=oh[:, j, :], rhs=msg[:, j, :],
                        start=bool(st2[t]), stop=bool(sp2[t]),
                        skip_group_check=True)

        # ---------- batched LayerNorm helper
        def batched_ln(pool, h_ap, W, F, g_t, be_t, out_ap, tg):
            sm = pool.tile([P, W], f32, tag=tg + "sm")
            nc.vector.tensor_reduce(out=sm[:], in_=h_ap, op=Alu.add, axis=AX)
            mu = pool.tile([P, W], f32, tag=tg + "mu")
            nc.vector.tensor_scalar(out=mu[:], in0=sm[:], scalar1=1.0 / F,
                                    scalar2=None, op0=Alu.mult)
            zc = pool.tile([P, W, F], bf16, tag=tg + "zc")
            nc.vector.tensor_tensor(out=zc[:, :, :], in0=h_ap,
                                    in1=bc_inner(mu[:], F), op=Alu.subtract)
            sq = pool.tile([P, W, F], bf16, tag=tg + "sq")
            nc.scalar.activation(sq[:], zc[:], Act.Square, scale=1.0)
            ssq = pool.tile([P, W], f32, tag=tg + "ss")
            nc.vector.tensor_reduce(out=ssq[:], in_=sq[:], op=Alu.add, axis=AX)
            Av = pool.tile([P, W], f32, tag=tg + "Av")
            nc.scalar.activation(Av[:], ssq[:], Act.Sqrt, bias=epst[:],
                                 scale=1.0 / F)
            nc.vector.reciprocal(Av[:], Av[:])
            t1 = pool.tile([P, W, F], bf16, tag=tg + "t1")
            nc.vector.tensor_tensor(out=t1[:, :, :], in0=zc[:, :, :],
                                    in1=bc_inner(Av[:], F), op=Alu.mult)
            t2 = pool.tile([P, W, F], bf16, tag=tg + "t2")
            nc.vector.tensor_tensor(out=t2[:, :, :], in0=t1[:, :, :],
                                    in1=bc_mid(g_t[:, 0:F], W), op=Alu.mult)
            nc.vector.tensor_tensor(out=out_ap, in0=t2[:, :, :],
                                    in1=bc_mid(be_t[:, 0:F], W), op=Alu.add)

        with tc.tile_pool(name="prep7", bufs=1) as pr7:
            posl_t = pr7.tile([P, NW, 3], f32, tag="posl")
            nc.sync.dma_start(out=posl_t[:], in_=pos_nm[:, :, :])
            nc.vector.tensor_copy(poolrhs[:, :, 64:67], posl_t[:])
            nc.vector.memset(poolrhs[:, :, 67:68], 1.0)

        CH = [(0, CMID, 0, GSPL1, 0, GSPL2), (CMID, NW, GSPL1, G1, GSPL2, G2)]
        with tc.tile_pool(name="acc1p", bufs=1, space="PSUM") as accp, \
             tc.tile_pool(name="am", bufs=2) as am, \
             tc.tile_pool(name="c1e", bufs=3) as c1pool, \
             tc.tile_pool(name="c1g", bufs=6) as c1gpool:
            for (clo, chi, c1a, c1b, g2a, g2b) in CH:
                acc = accp.tile([P, CMID * 64], f32, space="PSUM", tag="acc")
                with tc.tile_pool(name="c1z", bufs=2, space="PSUM") as zpool, \
                     tc.tile_pool(name="c1b", bufs=1, space="PSUM") as bcpool:
                    conv1_groups(c1pool, c1gpool, zpool, bcpool, acc,
                                 c1a, c1b, clo)
                def emit_p1b():
                    with tc.tile_pool(name="locp2", bufs=2) as locp, \
                         tc.tile_pool(name="locps2", bufs=1,
                                      space="PSUM") as locps:
                        for g in range(NGL):
                            jmax = min(TB, NW - g * TB)
                            xin = locp.tile([34, TB * P], bf16, tag="lxin")
                            nc.scalar.dma_start(
                                out=xin[:, 0:jmax * P],
                                in_=xcat_locT[:, g * TB * P:
                                              g * TB * P + jmax * P])
                            ps = locps.tile([P, TB * 128], f32, space="PSUM",
                                            tag="lps")
                            for j in range(jmax):
                                nc.tensor.matmul(
                                    ps[:, j * 128:(j + 1) * 128],
                                    lhsT=xin[:, j * P:(j + 1) * P],
                                    rhs=w_pro_b[:], start=True, stop=True,
                                    skip_group_check=True)
                            psv = ps[:].rearrange("p (t c) -> p t c", c=128)
                            nc.vector.tensor_copy(
                                a_loc[:, g * TB:g * TB + jmax, :],
                                psv[:, 0:jmax, 0:64])
                            nc.vector.tensor_tensor(
                                out=selfgcn[:, g * TB:g * TB + jmax, :],
                                in0=psv[:, 0:jmax, 64:128],
                                in1=bc_inner(
                                    dinvl_sq[:, g * TB:g * TB + jmax], 64),
                                op=Alu.mult)
                        for g in range(G2):
                            ein = locp.tile([12, TB * P], bf16, tag="ein")
                            nc.scalar.dma_start(
                                out=ein[:],
                                in_=eaT[:, g * TB * P:(g + 1) * TB * P])
                            ps = locps.tile([P, TB * 64], f32, space="PSUM",
                                            tag="qps")
                            for j in range(TB):
                                nc.tensor.matmul(
                                    ps[:, j * 64:(j + 1) * 64],
                                    lhsT=ein[:, j * P:(j + 1) * P],
                                    rhs=w_ea_b[:], start=True, stop=True,
                                    skip_group_check=True)
                            nc.scalar.activation(
                                q_all[:, g * TB:(g + 1) * TB, :],
                                ps[:].rearrange("p (t c) -> p t c", c=64),
                                Act.Copy, scale=1.0)
                    emit_off()
                with tc.tile_pool(name="g2e", bufs=3) as pool, \
                     tc.tile_pool(name="g2g", bufs=6) as gpool:
                    if clo == 0:
                        emit_p1b()
                    gcn_groups(pool, gpool, acc, g2a, g2b, clo)
                accv = acc[:].rearrange("p (w c) -> p w c", c=64)
                nc.vector.tensor_tensor(out=stage[:, clo:chi, :],
                                        in0=accv[:, 0:chi - clo, :],
                                        in1=off[:, clo:chi, :], op=Alu.add)
                # P7 (atom MLP) for this chunk — overlaps the next chunk's
                # gathers: nothing here touches the gpsimd engine.
                nwc = chi - clo
                with tc.tile_pool(name="amps", bufs=2, space="PSUM") as amps:
                    ngc = nwc // TB + (1 if nwc % TB else 0)
                    for gg in range(ngc):
                        jmax = min(TB, nwc - gg * TB)
                        hps = amps.tile([P, TB * 64], f32, space="PSUM",
                                        tag="hps")
                        for j in range(jmax):
                            w = clo + gg * TB + j
                            tp = amps.tile([64, P], bf16, space="PSUM",
                                           tag="atp")
                            nc.tensor.transpose(tp[:], stage[:, w, :],
                                                ident[:])
                            tps = am.tile([64, P], bf16, tag="atps")
                            nc.vector.tensor_copy(tps[:], tp[:])
                            nc.tensor.matmul(hps[:, j * 64:(j + 1) * 64],
                                             lhsT=tps[:], rhs=w_ae_b[:],
                                             start=True, stop=True,
                                             skip_group_check=True)
                        hv = hps[:].rearrange("p (t c) -> p t c", c=64)
                        nc.vector.tensor_tensor(
                            out=h_all[:, clo + gg * TB:clo + gg * TB + jmax, :],
                            in0=hv[:, 0:jmax, :], in1=bc_mid(b_ae_t[:], jmax),
                            op=Alu.add)
                    nc.scalar.activation(h_all[:, clo:chi, :],
                                         h_all[:, clo:chi, :], Act.Relu,
                                         scale=1.0)
                    batched_ln(am, h_all[:, clo:chi, :], nwc, 64, g_ae_t,
                               be_ae_t, poolrhs[:, clo:chi, 0:64], "aln")

        # ---------- P8: pooling into PSUM residue accumulator; P9: res MLP
        with tc.tile_pool(name="paccp", bufs=1, space="PSUM") as paccp, \
             tc.tile_pool(name="pw", bufs=3) as pw, \
             tc.tile_pool(name="rmps", bufs=1, space="PSUM") as rmps:
            pacc = paccp.tile([P, RW * 68], f32, space="PSUM")
            for g in range(NGL):
                jmax = min(TB, NW - g * TB)
                ohall = pw.tile([P, TB, P], bf16, tag="pohall")
                nc.vector.tensor_tensor(
                    out=ohall[:, 0:jmax, :], in0=bc_mid(iotab[:], jmax),
                    in1=bc_inner(rrelb[:, g * TB:g * TB + jmax], P),
                    op=Alu.is_equal)
                for j in range(jmax):
                    w = g * TB + j
                    rw_ = tile_rw[w]
                    nc.tensor.matmul(
                        pacc[:, rw_ * 68:(rw_ + 1) * 68],
                        lhsT=ohall[:, j, :], rhs=poolrhs[:, w, :],
                        start=bool(stp_f[w]), stop=bool(spp_f[w]),
                        skip_group_check=True)
            cntm = pw.tile([P, RW], f32, tag="cntm")
            for w in range(RW):
                nc.vector.tensor_scalar_max(
                    out=cntm[:, w:w + 1],
                    in0=pacc[:, w * 68 + 67:w * 68 + 68], scalar1=1.0)
            rec = pw.tile([P, RW], f32, tag="rec")
            nc.vector.reciprocal(rec[:], cntm[:])
            for w in range(RW):
                nc.vector.tensor_scalar(
                    out=resdat_sb[:, w, 64:67],
                    in0=pacc[:, w * 68 + 64:w * 68 + 67],
                    scalar1=rec[:, w:w + 1], scalar2=None, op0=Alu.mult)
            paccv = pacc[:].rearrange("p (w c) -> p w c", c=68)
            rsum = pw.tile([P, RW, 64], bf16, tag="rsum")
            nc.vector.tensor_copy(rsum[:], paccv[:, :, 0:64])
            hps = rmps.tile([P, RW * 64], f32, space="PSUM", tag="rhps")
            for w in range(RW):
                tp = rmps.tile([64, P], bf16, space="PSUM", tag="rtp")
                nc.tensor.transpose(tp[:], rsum[:, w, :], ident[:])
                tps = pw.tile([64, P], bf16, tag="rtps")
                nc.vector.tensor_copy(tps[:], tp[:])
                nc.tensor.matmul(hps[:, w * 64:(w + 1) * 64], lhsT=tps[:],
